# revision 32
# baseline (speedup 1.0000x reference)
"""Distillation loss (chunked KL + CE) on 8 Trainium2 NeuronCores — v7.

v6 (367us) made the Activation engine the only wall: 3 exp passes per
segment (e_t, e_s, e_ce) at 7.04us each, 48 instrs = 338us busy, with
DVE at 271us and Pool/PE idle.

v7 load-balances ALL THREE wide engines by giving each of the 16
segments (tile q of 128 tokens x chunk k of 8000 vocab) one of three
flavors:

  A  (5 segs): ACT A1,A2,A3 (Zu, Zv, Zce accums);  Pool: W1,W2 stt
  B1 (2 segs): ACT A1,A2; DVE: m2,m4,Zce-chain;    Pool: W1,W2 stt
  B2 (9 segs): ACT A1,A2; DVE: W1,W2,m2,m4;        Pool: Zce stt

where per segment (t', s' = logits/4 in fp8):
  A1: et  = exp(0.8 t') f8    accum -> Zu        (e_t = exp(t/5))
  A2: es  = exp(0.8 s') bf16  accum -> Zv
  A3: junk= exp(4 s')   bf16  accum -> Zce       (= sum exp(s), A only)
  W1: stt (et byp) mult t'    accum -> W1   [W = 4*(W1 - W2)]
  W2: stt (et byp) mult s'    accum -> W2   (no fp8-rounded t-s diff)
  m2 = es*es (bf16 2x tt), m4 = m2*m2, Zce = stt (m4 byp) mult es accum
       (= sum es^5 = sum exp(s) via 3 bf16 roundings, B only)

Engine busy/core: ACT 5*21.1+11*14.1 = 261us, DVE 2*17+9*25.4 = 263us,
Pool 5*22.2+2*22.2+9*11.1 = 255us -> balanced ~263us wall vs v6's 338.

Pipelining: segment-granular loads into a 3-deep ring of [128, 16000]
f8 seg buffers; et/es scratch 3-deep, m2/m4 2-deep (indexed by B-seg
ordinal). Semaphores: dTS +16/load, aE +1/ACT op, vD +1/DVE op,
pP +1/Pool op; prefix-sum arrays give exact wait values per segment.
Key transitive edges: A1(g) waits {dTS(g), vD/pP after seg g-3} which
frees et[g%3]/es[g%3]; DVE/Pool ops wait on aE for A1/A2 of their own
segment (implying the load); loads wait all three engines past seg g-3.
"""

from contextlib import ExitStack

import numpy as np

import concourse.bass as bass
import concourse.mybir as mybir
from concourse.bass_utils import run_bass_kernel_spmd

ALPHA = 0.7
TEMP = 5.0
PAD_ID = 0
NUM_CHUNKS = 4

N_CORES = 8
B, S, V = 2, 2048, 32000
TOK = B * S                      # 4096 tokens total
TPC = TOK // N_CORES             # 512 tokens per core
P = 128                          # SBUF partitions
Q = TPC // P                     # 4 token tiles per core (128 tokens each)
K = NUM_CHUNKS                   # 4 segments per tile
G = Q * K                        # 16 segments per core
CHW = V // NUM_CHUNKS            # 8000
PRESCALE = 0.25                  # host multiplies logits by this before fp8

F8 = mybir.dt.float8e3
BF16 = mybir.dt.bfloat16
F32 = mybir.dt.float32
EXP = mybir.ActivationFunctionType.Exp
MULT = mybir.AluOpType.mult
SUB = mybir.AluOpType.subtract
BYPASS = mybir.AluOpType.bypass

# Flavor schedule (all ops verified compilable: Pool does only plain
# tensor_tensor; stt+accum lives on DVE; exp+accum on ACT):
#   A : ACT A1,A2,A3; Pool D=t-s;       DVE W-stt
#   AD: ACT A1,A2,A3;                   DVE W1,W2 stt (no D)
#   B : ACT A1,A2;    Pool D;           DVE m2,m4,Zstt,W
#   C : ACT A1,A2;    Pool D, m2;       DVE m4,Zstt,W
FLAV = ['B', 'B', 'A', 'B', 'A', 'B', 'A', 'B', 'B', 'A',
        'B', 'A', 'B', 'A', 'A', 'A']
assert len(FLAV) == G

A_FLAVS = ('A', 'AD')

# acc column layout: [Zu 0:16 | Zv 16:32 | W 32:48 | W2(AD) 48:64 | Zce 64:80]
NSTAT = 80


def _build_nc(repeat=1, flav_order=None):
    nc = bass.Bass()
    ts = nc.dram_tensor("ts", [2, TPC, V], F8, kind="ExternalInput")
    st = nc.dram_tensor("stats", [P, NSTAT], F32, kind="ExternalOutput")

    ng = G * repeat
    base_flav = flav_order if flav_order is not None else FLAV
    flav = [base_flav[g % G] for g in range(ng)]
    # B-seg ordinal (segments with a Zce chain) for es/m24 rings
    bidx = []
    b = 0
    for f in flav:
        bidx.append(b)
        if f not in A_FLAVS:
            b += 1
    bseg = [g for g in range(ng) if flav[g] not in A_FLAVS]

    # --- engine op streams (program order) + 1-based position maps -----
    # ACT: A1 runs one segment ahead of A2/A3, but the cadence starts at
    # seg 1 so A2(0) lands second and the DVE/Pool chains ramp early.
    act_stream = [('A1', 0), ('A2', 0)]
    if flav[0] in A_FLAVS:
        act_stream.append(('A3', 0))
    if ng > 1:
        act_stream.append(('A1', 1))
    for g in range(1, ng):
        if g + 1 < ng:
            act_stream.append(('A1', g + 1))
        act_stream.append(('A2', g))
        if flav[g] in A_FLAVS:
            act_stream.append(('A3', g))

    # Pool: D(g) asap; for C segs also m2(g) (after A2(g)).
    pool_stream = []
    for g in range(ng):
        if flav[g] != 'AD':
            pool_stream.append(('D', g))
        if flav[g] == 'C':
            pool_stream.append(('m2', g))

    # DVE: A: [W]; AD: [W1, W2]; B: [m2, m4, Zstt, W]; C: [W, m4, Zstt]
    dve_stream = []
    for g in range(ng):
        f = flav[g]
        if f == 'A':
            dve_stream.append(('W', g))
        elif f == 'AD':
            dve_stream.append(('W1', g))
            dve_stream.append(('W2', g))
        elif f == 'B':
            dve_stream.extend([('m2', g), ('m4', g), ('Zstt', g), ('W', g)])
        else:  # C
            dve_stream.extend([('W', g), ('m4', g), ('Zstt', g)])

    def posmap(stream):
        m = {}
        for i, op in enumerate(stream):
            m[op] = i + 1
        return m

    aP = posmap(act_stream)
    pPos = posmap(pool_stream)
    vPos = posmap(dve_stream)
    aE_total, pP_total, vD_total = len(act_stream), len(pool_stream), len(dve_stream)

    def after_last_act(g):      # last ACT op of seg g
        return aP[('A3', g)] if flav[g] in A_FLAVS else aP[('A2', g)]

    def after_last_dve_w(g):    # the W product(s) of seg g on DVE
        return vPos[('W2', g)] if flav[g] == 'AD' else vPos[('W', g)]

    def waits_readers_of_bufs(g):
        """t/s buffer of seg g is free when ACT (A2/A3), Pool (D) and
        DVE (W, in-place over the t-region) are all past seg g."""
        aw = after_last_act(g)
        vw = after_last_dve_w(g)
        pw = pPos[('D', g)] if flav[g] != 'AD' else 0
        return aw, vw, pw

    def waits_readers_of_et(g):
        return after_last_dve_w(g), 0

    def waits_readers_of_es(g):
        """Readers of the es slot written at seg g (B/C chains only)."""
        vw, pw = 0, 0
        if flav[g] == 'B':
            vw = vPos[('Zstt', g)]
        elif flav[g] == 'C':
            vw = vPos[('Zstt', g)]
            pw = pPos[('m2', g)]
        return vw, pw

    with ExitStack() as ctx:
        bufs = [
            ctx.enter_context(nc.sbuf_tensor(f"buf{i}", [P, 2 * CHW], F8))
            for i in range(5)
        ]
        et = [
            ctx.enter_context(nc.sbuf_tensor(f"et{i}", [P, CHW], F8))
            for i in range(4)
        ]
        es = [
            ctx.enter_context(nc.sbuf_tensor(f"es{i}", [P, CHW], BF16))
            for i in range(3)
        ]
        m24 = [
            ctx.enter_context(nc.sbuf_tensor(f"m24_{i}", [P, CHW], BF16))
            for i in range(2)
        ]
        acc = ctx.enter_context(nc.sbuf_tensor("acc", [P, NSTAT], F32))
        dTS = ctx.enter_context(nc.semaphore("dTS"))
        aE = ctx.enter_context(nc.semaphore("aE"))
        vD = ctx.enter_context(nc.semaphore("vD"))
        pP = ctx.enter_context(nc.semaphore("pP"))
        out_sem = ctx.enter_context(nc.semaphore("out_sem"))
        block = ctx.enter_context(nc.Block())

        def tseg(g):
            return bufs[g % 5][:, 0:CHW]

        def sseg(g):
            return bufs[g % 5][:, CHW:2 * CHW]

        def etb(g):
            return et[g % 4][:, :]

        def esb(g):
            return es[bidx[g] % 3][:, :]

        def m24b(g):
            return m24[bidx[g] % 2][:, :]

        def col(base, g):
            c = base * G + (g % G)
            return acc[:, c:c + 1]

        @block.sync
        def _(sync):
            for g in range(ng):
                q, k = (g % G) // K, g % K
                if g >= 5:
                    aw, vw, pw = waits_readers_of_bufs(g - 5)
                    sync.wait_ge(aE, aw)
                    sync.wait_ge(vD, vw)
                    if pw > 0:
                        sync.wait_ge(pP, pw)
                r0 = q * P
                src = ts[:, r0:r0 + P, k * CHW:(k + 1) * CHW].rearrange(
                    "a p v -> p a v")
                dst = bufs[g % 5][:].rearrange("p (a v) -> p a v", a=2)
                if g == 0:
                    # split the very first load: t-half lands ~3us sooner
                    # so A1(0) starts earlier (halves inc +8 each)
                    sync.dma_start(out=dst[:, 0:1, :],
                                   in_=src[:, 0:1, :]).then_inc(dTS, 16)
                    sync.dma_start(out=dst[:, 1:2, :],
                                   in_=src[:, 1:2, :]).then_inc(dTS, 16)
                else:
                    sync.dma_start(out=dst, in_=src).then_inc(dTS, 16)
            sync.wait_ge(aE, aE_total)
            sync.wait_ge(vD, vD_total)
            sync.wait_ge(pP, pP_total)
            sync.dma_start(out=st[:, :], in_=acc[:]).then_inc(out_sem, 16)
            sync.wait_ge(out_sem, 16)

        @block.scalar
        def _(scalar):
            for kind, g in act_stream:
                if kind == 'A1':
                    scalar.wait_ge(dTS, 16 if g == 0 else 16 * (g + 2))
                    if g >= 4:
                        vw, pw = waits_readers_of_et(g - 4)
                        scalar.wait_ge(vD, vw)
                    nc.scalar.activation(
                        etb(g), tseg(g), EXP, bias=0.0, scale=0.8,
                        accum_out=col(0, g),
                    ).then_inc(aE, 1)
                elif kind == 'A2':
                    if g == 0:
                        scalar.wait_ge(dTS, 32)  # s-half of split load 0
                    bprev = bidx[g] - 3   # es ring depth 3
                    if bprev >= 0:
                        vw, pw = waits_readers_of_es(bseg[bprev])
                        if vw > 0:
                            scalar.wait_ge(vD, vw)
                        if pw > 0:
                            scalar.wait_ge(pP, pw)
                    nc.scalar.activation(
                        esb(g), sseg(g), EXP, bias=0.0, scale=0.8,
                        accum_out=col(1, g),
                    ).then_inc(aE, 1)
                else:  # A3
                    nc.scalar.activation(
                        esb(g), sseg(g), EXP, bias=0.0, scale=4.0,
                        accum_out=col(4, g),
                    ).then_inc(aE, 1)

        @block.vector
        def _(vector):
            for kind, g in dve_stream:
                if kind == 'W':
                    # (D byp 1) mult e_t, in place over the t-region.
                    # D(g) on Pool implies A1(g) (its own wait) and load.
                    vector.wait_ge(pP, pPos[('D', g)])
                    nc.vector.scalar_tensor_tensor(
                        out=tseg(g), in0=tseg(g), scalar=1.0,
                        in1=etb(g), op0=BYPASS, op1=MULT,
                        accum_out=col(2, g),
                    ).then_inc(vD, 1)
                elif kind == 'W1':
                    vector.wait_ge(aE, aP[('A1', g)])
                    nc.vector.scalar_tensor_tensor(
                        out=tseg(g), in0=etb(g), scalar=1.0,
                        in1=tseg(g), op0=BYPASS, op1=MULT,
                        accum_out=col(2, g),
                    ).then_inc(vD, 1)
                elif kind == 'W2':
                    nc.vector.scalar_tensor_tensor(
                        out=tseg(g), in0=etb(g), scalar=1.0,
                        in1=sseg(g), op0=BYPASS, op1=MULT,
                        accum_out=col(3, g),
                    ).then_inc(vD, 1)
                elif kind == 'm2':   # B only (C's m2 is on Pool)
                    vector.wait_ge(aE, aP[('A2', g)])
                    bp2 = bidx[g] - 2
                    if bp2 >= 0 and flav[bseg[bp2]] == 'C':
                        vector.wait_ge(pP, pPos[('m2', bseg[bp2])])
                    nc.vector.tensor_tensor(
                        out=m24b(g), in0=esb(g), in1=esb(g), op=MULT,
                    ).then_inc(vD, 1)
                elif kind == 'm4':
                    if flav[g] == 'C':
                        vector.wait_ge(pP, pPos[('m2', g)])
                    nc.vector.tensor_tensor(
                        out=m24b(g), in0=m24b(g), in1=m24b(g), op=MULT,
                    ).then_inc(vD, 1)
                else:  # Zstt
                    nc.vector.scalar_tensor_tensor(
                        out=m24b(g), in0=m24b(g), scalar=1.0,
                        in1=esb(g), op0=BYPASS, op1=MULT,
                        accum_out=col(4, g),
                    ).then_inc(vD, 1)

        @block.gpsimd
        def _(gp):
            for kind, g in pool_stream:
                if kind == 'D':
                    # D = t - s in place over the t-region (A1(g) read t)
                    if g == 0:
                        gp.wait_ge(dTS, 32)  # s-half of split load 0
                    gp.wait_ge(aE, aP[('A1', g)])
                    nc.gpsimd.tensor_tensor(
                        out=tseg(g), in0=tseg(g), in1=sseg(g), op=SUB,
                    ).then_inc(pP, 1)
                else:  # m2 for C segs
                    gp.wait_ge(aE, aP[('A2', g)])
                    bprev = bidx[g] - 2
                    if bprev >= 0:
                        f2 = flav[bseg[bprev]]
                        if f2 in ('B', 'C'):
                            gp.wait_ge(vD, vPos[('Zstt', bseg[bprev])])
                    nc.gpsimd.tensor_tensor(
                        out=m24b(g), in0=esb(g), in1=esb(g), op=MULT,
                    ).then_inc(pP, 1)

    return nc


_NC_CACHE = {}
last_results = None


def _get_nc(repeat=1):
    if repeat not in _NC_CACHE:
        _NC_CACHE[repeat] = _build_nc(repeat)
    return _NC_CACHE[repeat]


def _combine(results, s_full, lab):
    """Host-side float64 reduction of per-core [128, 80] stats -> loss."""
    # token = c*TPC + q*P + p ; segment g = 4q + chunk j
    w = np.empty((TOK, NUM_CHUNKS))
    zu = np.empty((TOK, NUM_CHUNKS))
    zv = np.empty((TOK, NUM_CHUNKS))
    zce = np.empty(TOK)

    def tokmajor(block):  # [P, G] -> [TPC, NUM_CHUNKS] in token order
        return block.reshape(P, Q, K).transpose(1, 0, 2).reshape(TPC, K)

    for c, r in enumerate(results):
        a = r["stats"].astype(np.float64)          # [128, 80]
        sl = slice(c * TPC, (c + 1) * TPC)
        zu[sl] = tokmajor(a[:, 0:G])
        zv[sl] = tokmajor(a[:, G:2 * G])
        wc = a[:, 2 * G:3 * G].copy()
        ad = np.array([f == 'AD' for f in FLAV])[None, :]
        wc = np.where(ad, wc - a[:, 3 * G:4 * G], wc)  # AD segs: W1 - W2
        w[sl] = tokmajor(wc)
        zce[sl] = tokmajor(a[:, 4 * G:5 * G]).sum(axis=1)

    # W stored = sum e_t*(t-s)/4 -> true sum e_t*(t-s) = 4*W
    kl = (4.0 * w) / (TEMP * zu) + np.log(zv) - np.log(zu)
    total_kl = kl.sum() * (TEMP * TEMP) * (CHW / V) / B

    s_label = s_full[np.arange(TOK), lab].astype(np.float64)
    nll = np.log(zce) - s_label
    valid = lab != PAD_ID
    n_valid = max(int(valid.sum()), 1)
    ce = float(nll[valid].sum()) / n_valid

    return ALPHA * total_kl + (1.0 - ALPHA) * ce


def kernel(student_logits, teacher_logits, labels):
    global last_results
    np_f8 = mybir.dt.np(F8)
    s_full = np.asarray(student_logits, dtype=np.float32).reshape(TOK, V)
    t_full = np.asarray(teacher_logits, dtype=np.float32).reshape(TOK, V)
    lab = np.asarray(labels).reshape(TOK).astype(np.int64)
    s_f8 = (s_full * PRESCALE).astype(np_f8)
    t_f8 = (t_full * PRESCALE).astype(np_f8)

    nc = _get_nc()
    in_maps = []
    for c in range(N_CORES):
        ts = np.ascontiguousarray(np.stack(
            [t_f8[c * TPC:(c + 1) * TPC], s_f8[c * TPC:(c + 1) * TPC]], axis=0))
        in_maps.append({"ts": ts})
    last_results = run_bass_kernel_spmd(nc, in_maps, core_ids=list(range(N_CORES)))
    loss = _combine(last_results.results, s_full, lab)
    return np.array(loss, dtype=np.float32)


# revision 33
# speedup vs baseline: 1.0036x; 1.0036x over previous
"""Distillation loss (chunked KL + CE) on 8 Trainium2 NeuronCores — v7.

v6 (367us) made the Activation engine the only wall: 3 exp passes per
segment (e_t, e_s, e_ce) at 7.04us each, 48 instrs = 338us busy, with
DVE at 271us and Pool/PE idle.

v7 load-balances ALL THREE wide engines by giving each of the 16
segments (tile q of 128 tokens x chunk k of 8000 vocab) one of three
flavors:

  A  (5 segs): ACT A1,A2,A3 (Zu, Zv, Zce accums);  Pool: W1,W2 stt
  B1 (2 segs): ACT A1,A2; DVE: m2,m4,Zce-chain;    Pool: W1,W2 stt
  B2 (9 segs): ACT A1,A2; DVE: W1,W2,m2,m4;        Pool: Zce stt

where per segment (t', s' = logits/4 in fp8):
  A1: et  = exp(0.8 t') f8    accum -> Zu        (e_t = exp(t/5))
  A2: es  = exp(0.8 s') bf16  accum -> Zv
  A3: junk= exp(4 s')   bf16  accum -> Zce       (= sum exp(s), A only)
  W1: stt (et byp) mult t'    accum -> W1   [W = 4*(W1 - W2)]
  W2: stt (et byp) mult s'    accum -> W2   (no fp8-rounded t-s diff)
  m2 = es*es (bf16 2x tt), m4 = m2*m2, Zce = stt (m4 byp) mult es accum
       (= sum es^5 = sum exp(s) via 3 bf16 roundings, B only)

Engine busy/core: ACT 5*21.1+11*14.1 = 261us, DVE 2*17+9*25.4 = 263us,
Pool 5*22.2+2*22.2+9*11.1 = 255us -> balanced ~263us wall vs v6's 338.

Pipelining: segment-granular loads into a 3-deep ring of [128, 16000]
f8 seg buffers; et/es scratch 3-deep, m2/m4 2-deep (indexed by B-seg
ordinal). Semaphores: dTS +16/load, aE +1/ACT op, vD +1/DVE op,
pP +1/Pool op; prefix-sum arrays give exact wait values per segment.
Key transitive edges: A1(g) waits {dTS(g), vD/pP after seg g-3} which
frees et[g%3]/es[g%3]; DVE/Pool ops wait on aE for A1/A2 of their own
segment (implying the load); loads wait all three engines past seg g-3.
"""

from contextlib import ExitStack

import numpy as np

import concourse.bass as bass
import concourse.mybir as mybir
from concourse.bass_utils import run_bass_kernel_spmd

ALPHA = 0.7
TEMP = 5.0
PAD_ID = 0
NUM_CHUNKS = 4

N_CORES = 8
B, S, V = 2, 2048, 32000
TOK = B * S                      # 4096 tokens total
TPC = TOK // N_CORES             # 512 tokens per core
P = 128                          # SBUF partitions
Q = TPC // P                     # 4 token tiles per core (128 tokens each)
K = NUM_CHUNKS                   # 4 segments per tile
G = Q * K                        # 16 segments per core
CHW = V // NUM_CHUNKS            # 8000
PRESCALE = 0.25                  # host multiplies logits by this before fp8

F8 = mybir.dt.float8e3
BF16 = mybir.dt.bfloat16
F32 = mybir.dt.float32
EXP = mybir.ActivationFunctionType.Exp
MULT = mybir.AluOpType.mult
SUB = mybir.AluOpType.subtract
BYPASS = mybir.AluOpType.bypass

# Flavor schedule (all ops verified compilable: Pool does only plain
# tensor_tensor; stt+accum lives on DVE; exp+accum on ACT):
#   A : ACT A1,A2,A3; Pool D=t-s;       DVE W-stt
#   AD: ACT A1,A2,A3;                   DVE W1,W2 stt (no D)
#   B : ACT A1,A2;    Pool D;           DVE m2,m4,Zstt,W
#   C : ACT A1,A2;    Pool D, m2;       DVE m4,Zstt,W
FLAV = ['B', 'B', 'A', 'B', 'A', 'B', 'A', 'B', 'B', 'A',
        'B', 'A', 'B', 'A', 'A', 'A']
assert len(FLAV) == G

A_FLAVS = ('A', 'AD')

# acc column layout: [Zu 0:16 | Zv 16:32 | W 32:48 | W2(AD) 48:64 | Zce 64:80]
NSTAT = 80


def _build_nc(repeat=1, flav_order=None):
    nc = bass.Bass()
    ts = nc.dram_tensor("ts", [2, TPC, V], F8, kind="ExternalInput")
    st = nc.dram_tensor("stats", [P, NSTAT], F32, kind="ExternalOutput")

    ng = G * repeat
    base_flav = flav_order if flav_order is not None else FLAV
    flav = [base_flav[g % G] for g in range(ng)]
    # B-seg ordinal (segments with a Zce chain) for es/m24 rings
    bidx = []
    b = 0
    for f in flav:
        bidx.append(b)
        if f not in A_FLAVS:
            b += 1
    bseg = [g for g in range(ng) if flav[g] not in A_FLAVS]

    # --- engine op streams (program order) + 1-based position maps -----
    # ACT: A1 runs one segment ahead of A2/A3, but the cadence starts at
    # seg 1 so A2(0) lands second and the DVE/Pool chains ramp early.
    act_stream = [('A1a', 0), ('A1b', 0), ('A2', 0)]
    if flav[0] in A_FLAVS:
        act_stream.append(('A3', 0))
    if ng > 1:
        act_stream.append(('A1', 1))
    for g in range(1, ng):
        if g + 1 < ng:
            act_stream.append(('A1', g + 1))
        act_stream.append(('A2', g))
        if flav[g] in A_FLAVS:
            act_stream.append(('A3', g))

    # Pool: D(g) asap; for C segs also m2(g) (after A2(g)).
    pool_stream = []
    for g in range(ng):
        if flav[g] != 'AD':
            pool_stream.append(('D', g))
        if flav[g] == 'C':
            pool_stream.append(('m2', g))

    # DVE: A: [W]; AD: [W1, W2]; B: [m2, m4, Zstt, W]; C: [W, m4, Zstt]
    dve_stream = []
    for g in range(ng):
        f = flav[g]
        if f == 'A':
            dve_stream.append(('W', g))
        elif f == 'AD':
            dve_stream.append(('W1', g))
            dve_stream.append(('W2', g))
        elif f == 'B':
            dve_stream.extend([('m2', g), ('m4', g), ('Zstt', g), ('W', g)])
        else:  # C
            dve_stream.extend([('W', g), ('m4', g), ('Zstt', g)])

    def posmap(stream):
        m = {}
        for i, op in enumerate(stream):
            m[op] = i + 1
        return m

    aP = posmap(act_stream)
    pPos = posmap(pool_stream)
    vPos = posmap(dve_stream)
    aE_total, pP_total, vD_total = len(act_stream), len(pool_stream), len(dve_stream)

    def a1_pos(g):
        return aP[('A1b', 0)] if g == 0 else aP[('A1', g)]

    def after_last_act(g):      # last ACT op of seg g
        return aP[('A3', g)] if flav[g] in A_FLAVS else aP[('A2', g)]

    def after_last_dve_w(g):    # the W product(s) of seg g on DVE
        return vPos[('W2', g)] if flav[g] == 'AD' else vPos[('W', g)]

    def waits_readers_of_bufs(g):
        """t/s buffer of seg g is free when ACT (A2/A3), Pool (D) and
        DVE (W, in-place over the t-region) are all past seg g."""
        aw = after_last_act(g)
        vw = after_last_dve_w(g)
        pw = pPos[('D', g)] if flav[g] != 'AD' else 0
        return aw, vw, pw

    def waits_readers_of_et(g):
        return after_last_dve_w(g), 0

    def waits_readers_of_es(g):
        """Readers of the es slot written at seg g (B/C chains only)."""
        vw, pw = 0, 0
        if flav[g] == 'B':
            vw = vPos[('Zstt', g)]
        elif flav[g] == 'C':
            vw = vPos[('Zstt', g)]
            pw = pPos[('m2', g)]
        return vw, pw

    with ExitStack() as ctx:
        bufs = [
            ctx.enter_context(nc.sbuf_tensor(f"buf{i}", [P, 2 * CHW], F8))
            for i in range(5)
        ]
        et = [
            ctx.enter_context(nc.sbuf_tensor(f"et{i}", [P, CHW], F8))
            for i in range(4)
        ]
        es = [
            ctx.enter_context(nc.sbuf_tensor(f"es{i}", [P, CHW], BF16))
            for i in range(3)
        ]
        m24 = [
            ctx.enter_context(nc.sbuf_tensor(f"m24_{i}", [P, CHW], BF16))
            for i in range(2)
        ]
        acc = ctx.enter_context(nc.sbuf_tensor("acc", [P, NSTAT], F32))
        dTS = ctx.enter_context(nc.semaphore("dTS"))
        aE = ctx.enter_context(nc.semaphore("aE"))
        vD = ctx.enter_context(nc.semaphore("vD"))
        pP = ctx.enter_context(nc.semaphore("pP"))
        out_sem = ctx.enter_context(nc.semaphore("out_sem"))
        block = ctx.enter_context(nc.Block())

        def tseg(g):
            return bufs[g % 5][:, 0:CHW]

        def sseg(g):
            return bufs[g % 5][:, CHW:2 * CHW]

        def etb(g):
            return et[g % 4][:, :]

        def esb(g):
            return es[bidx[g] % 3][:, :]

        def m24b(g):
            return m24[bidx[g] % 2][:, :]

        def col(base, g):
            c = base * G + (g % G)
            return acc[:, c:c + 1]

        @block.sync
        def _(sync):
            for g in range(ng):
                q, k = (g % G) // K, g % K
                if g >= 5:
                    aw, vw, pw = waits_readers_of_bufs(g - 5)
                    sync.wait_ge(aE, aw)
                    sync.wait_ge(vD, vw)
                    if pw > 0:
                        sync.wait_ge(pP, pw)
                r0 = q * P
                src = ts[:, r0:r0 + P, k * CHW:(k + 1) * CHW].rearrange(
                    "a p v -> p a v")
                dst = bufs[g % 5][:].rearrange("p (a v) -> p a v", a=2)
                if g == 0:
                    # first load in three pieces: t[0:4000], t[4000:8000],
                    # s -- so A1a can start ~4.6us in
                    h2 = CHW // 2
                    sync.dma_start(out=dst[:, 0:1, 0:h2],
                                   in_=src[:, 0:1, 0:h2]).then_inc(dTS, 16)
                    sync.dma_start(out=dst[:, 0:1, h2:CHW],
                                   in_=src[:, 0:1, h2:CHW]).then_inc(dTS, 16)
                    sync.dma_start(out=dst[:, 1:2, :],
                                   in_=src[:, 1:2, :]).then_inc(dTS, 16)
                else:
                    sync.dma_start(out=dst, in_=src).then_inc(dTS, 16)
            sync.wait_ge(aE, aE_total)
            sync.wait_ge(vD, vD_total)
            sync.wait_ge(pP, pP_total)
            sync.dma_start(out=st[:, :], in_=acc[:]).then_inc(out_sem, 16)
            sync.wait_ge(out_sem, 16)

        @block.scalar
        def _(scalar):
            HALF = CHW // 2
            for kind, g in act_stream:
                if kind == 'A1a':
                    scalar.wait_ge(dTS, 16)
                    nc.scalar.activation(
                        etb(0)[:, 0:HALF], tseg(0)[:, 0:HALF], EXP,
                        bias=0.0, scale=0.8, accum_out=col(0, 0),
                    ).then_inc(aE, 1)
                elif kind == 'A1b':
                    scalar.wait_ge(dTS, 32)
                    nc.scalar.activation(
                        etb(0)[:, HALF:CHW], tseg(0)[:, HALF:CHW], EXP,
                        bias=0.0, scale=0.8, accum_out=col(3, 0),
                    ).then_inc(aE, 1)
                elif kind == 'A1':
                    scalar.wait_ge(dTS, 16 * (g + 3))
                    if g >= 4:
                        vw, pw = waits_readers_of_et(g - 4)
                        scalar.wait_ge(vD, vw)
                    nc.scalar.activation(
                        etb(g), tseg(g), EXP, bias=0.0, scale=0.8,
                        accum_out=col(0, g),
                    ).then_inc(aE, 1)
                elif kind == 'A2':
                    if g == 0:
                        scalar.wait_ge(dTS, 48)  # s-part of split load 0
                    bprev = bidx[g] - 3   # es ring depth 3
                    if bprev >= 0:
                        vw, pw = waits_readers_of_es(bseg[bprev])
                        if vw > 0:
                            scalar.wait_ge(vD, vw)
                        if pw > 0:
                            scalar.wait_ge(pP, pw)
                    nc.scalar.activation(
                        esb(g), sseg(g), EXP, bias=0.0, scale=0.8,
                        accum_out=col(1, g),
                    ).then_inc(aE, 1)
                else:  # A3
                    nc.scalar.activation(
                        esb(g), sseg(g), EXP, bias=0.0, scale=4.0,
                        accum_out=col(4, g),
                    ).then_inc(aE, 1)

        @block.vector
        def _(vector):
            for kind, g in dve_stream:
                if kind == 'W':
                    # (D byp 1) mult e_t, in place over the t-region.
                    # D(g) on Pool implies A1(g) (its own wait) and load.
                    vector.wait_ge(pP, pPos[('D', g)])
                    nc.vector.scalar_tensor_tensor(
                        out=tseg(g), in0=tseg(g), scalar=1.0,
                        in1=etb(g), op0=BYPASS, op1=MULT,
                        accum_out=col(2, g),
                    ).then_inc(vD, 1)
                elif kind == 'W1':
                    vector.wait_ge(aE, a1_pos(g))
                    nc.vector.scalar_tensor_tensor(
                        out=tseg(g), in0=etb(g), scalar=1.0,
                        in1=tseg(g), op0=BYPASS, op1=MULT,
                        accum_out=col(2, g),
                    ).then_inc(vD, 1)
                elif kind == 'W2':
                    nc.vector.scalar_tensor_tensor(
                        out=tseg(g), in0=etb(g), scalar=1.0,
                        in1=sseg(g), op0=BYPASS, op1=MULT,
                        accum_out=col(3, g),
                    ).then_inc(vD, 1)
                elif kind == 'm2':   # B only (C's m2 is on Pool)
                    vector.wait_ge(aE, aP[('A2', g)])
                    bp2 = bidx[g] - 2
                    if bp2 >= 0 and flav[bseg[bp2]] == 'C':
                        vector.wait_ge(pP, pPos[('m2', bseg[bp2])])
                    nc.vector.tensor_tensor(
                        out=m24b(g), in0=esb(g), in1=esb(g), op=MULT,
                    ).then_inc(vD, 1)
                elif kind == 'm4':
                    if flav[g] == 'C':
                        vector.wait_ge(pP, pPos[('m2', g)])
                    nc.vector.tensor_tensor(
                        out=m24b(g), in0=m24b(g), in1=m24b(g), op=MULT,
                    ).then_inc(vD, 1)
                else:  # Zstt
                    nc.vector.scalar_tensor_tensor(
                        out=m24b(g), in0=m24b(g), scalar=1.0,
                        in1=esb(g), op0=BYPASS, op1=MULT,
                        accum_out=col(4, g),
                    ).then_inc(vD, 1)

        @block.gpsimd
        def _(gp):
            for kind, g in pool_stream:
                if kind == 'D':
                    # D = t - s in place over the t-region (A1(g) read t)
                    if g == 0:
                        gp.wait_ge(dTS, 48)  # s-part of split load 0
                    gp.wait_ge(aE, a1_pos(g))
                    nc.gpsimd.tensor_tensor(
                        out=tseg(g), in0=tseg(g), in1=sseg(g), op=SUB,
                    ).then_inc(pP, 1)
                else:  # m2 for C segs
                    gp.wait_ge(aE, aP[('A2', g)])
                    bprev = bidx[g] - 2
                    if bprev >= 0:
                        f2 = flav[bseg[bprev]]
                        if f2 in ('B', 'C'):
                            gp.wait_ge(vD, vPos[('Zstt', bseg[bprev])])
                    nc.gpsimd.tensor_tensor(
                        out=m24b(g), in0=esb(g), in1=esb(g), op=MULT,
                    ).then_inc(pP, 1)

    return nc


_NC_CACHE = {}
last_results = None


def _get_nc(repeat=1):
    if repeat not in _NC_CACHE:
        _NC_CACHE[repeat] = _build_nc(repeat)
    return _NC_CACHE[repeat]


def _combine(results, s_full, lab):
    """Host-side float64 reduction of per-core [128, 80] stats -> loss."""
    # token = c*TPC + q*P + p ; segment g = 4q + chunk j
    w = np.empty((TOK, NUM_CHUNKS))
    zu = np.empty((TOK, NUM_CHUNKS))
    zv = np.empty((TOK, NUM_CHUNKS))
    zce = np.empty(TOK)

    def tokmajor(block):  # [P, G] -> [TPC, NUM_CHUNKS] in token order
        return block.reshape(P, Q, K).transpose(1, 0, 2).reshape(TPC, K)

    for c, r in enumerate(results):
        a = r["stats"].astype(np.float64)          # [128, 80]
        sl = slice(c * TPC, (c + 1) * TPC)
        zub = a[:, 0:G].copy()
        zub[:, 0] += a[:, 3 * G]       # seg 0 A1 split: Zu = half a + half b
        zu[sl] = tokmajor(zub)
        zv[sl] = tokmajor(a[:, G:2 * G])
        wc = a[:, 2 * G:3 * G].copy()
        ad = np.array([f == 'AD' for f in FLAV])[None, :]
        wc = np.where(ad, wc - a[:, 3 * G:4 * G], wc)  # AD segs: W1 - W2
        w[sl] = tokmajor(wc)
        zce[sl] = tokmajor(a[:, 4 * G:5 * G]).sum(axis=1)

    # W stored = sum e_t*(t-s)/4 -> true sum e_t*(t-s) = 4*W
    kl = (4.0 * w) / (TEMP * zu) + np.log(zv) - np.log(zu)
    total_kl = kl.sum() * (TEMP * TEMP) * (CHW / V) / B

    s_label = s_full[np.arange(TOK), lab].astype(np.float64)
    nll = np.log(zce) - s_label
    valid = lab != PAD_ID
    n_valid = max(int(valid.sum()), 1)
    ce = float(nll[valid].sum()) / n_valid

    return ALPHA * total_kl + (1.0 - ALPHA) * ce


def kernel(student_logits, teacher_logits, labels):
    global last_results
    np_f8 = mybir.dt.np(F8)
    s_full = np.asarray(student_logits, dtype=np.float32).reshape(TOK, V)
    t_full = np.asarray(teacher_logits, dtype=np.float32).reshape(TOK, V)
    lab = np.asarray(labels).reshape(TOK).astype(np.int64)
    s_f8 = (s_full * PRESCALE).astype(np_f8)
    t_f8 = (t_full * PRESCALE).astype(np_f8)

    nc = _get_nc()
    in_maps = []
    for c in range(N_CORES):
        ts = np.ascontiguousarray(np.stack(
            [t_f8[c * TPC:(c + 1) * TPC], s_f8[c * TPC:(c + 1) * TPC]], axis=0))
        in_maps.append({"ts": ts})
    last_results = run_bass_kernel_spmd(nc, in_maps, core_ids=list(range(N_CORES)))
    loss = _combine(last_results.results, s_full, lab)
    return np.array(loss, dtype=np.float32)


# revision 34
# speedup vs baseline: 1.0048x; 1.0012x over previous
"""Distillation loss (chunked KL + CE) on 8 Trainium2 NeuronCores — v7.

v6 (367us) made the Activation engine the only wall: 3 exp passes per
segment (e_t, e_s, e_ce) at 7.04us each, 48 instrs = 338us busy, with
DVE at 271us and Pool/PE idle.

v7 load-balances ALL THREE wide engines by giving each of the 16
segments (tile q of 128 tokens x chunk k of 8000 vocab) one of three
flavors:

  A  (5 segs): ACT A1,A2,A3 (Zu, Zv, Zce accums);  Pool: W1,W2 stt
  B1 (2 segs): ACT A1,A2; DVE: m2,m4,Zce-chain;    Pool: W1,W2 stt
  B2 (9 segs): ACT A1,A2; DVE: W1,W2,m2,m4;        Pool: Zce stt

where per segment (t', s' = logits/4 in fp8):
  A1: et  = exp(0.8 t') f8    accum -> Zu        (e_t = exp(t/5))
  A2: es  = exp(0.8 s') bf16  accum -> Zv
  A3: junk= exp(4 s')   bf16  accum -> Zce       (= sum exp(s), A only)
  W1: stt (et byp) mult t'    accum -> W1   [W = 4*(W1 - W2)]
  W2: stt (et byp) mult s'    accum -> W2   (no fp8-rounded t-s diff)
  m2 = es*es (bf16 2x tt), m4 = m2*m2, Zce = stt (m4 byp) mult es accum
       (= sum es^5 = sum exp(s) via 3 bf16 roundings, B only)

Engine busy/core: ACT 5*21.1+11*14.1 = 261us, DVE 2*17+9*25.4 = 263us,
Pool 5*22.2+2*22.2+9*11.1 = 255us -> balanced ~263us wall vs v6's 338.

Pipelining: segment-granular loads into a 3-deep ring of [128, 16000]
f8 seg buffers; et/es scratch 3-deep, m2/m4 2-deep (indexed by B-seg
ordinal). Semaphores: dTS +16/load, aE +1/ACT op, vD +1/DVE op,
pP +1/Pool op; prefix-sum arrays give exact wait values per segment.
Key transitive edges: A1(g) waits {dTS(g), vD/pP after seg g-3} which
frees et[g%3]/es[g%3]; DVE/Pool ops wait on aE for A1/A2 of their own
segment (implying the load); loads wait all three engines past seg g-3.
"""

from contextlib import ExitStack

import numpy as np

import concourse.bass as bass
import concourse.mybir as mybir
from concourse.bass_utils import run_bass_kernel_spmd

ALPHA = 0.7
TEMP = 5.0
PAD_ID = 0
NUM_CHUNKS = 4

N_CORES = 8
B, S, V = 2, 2048, 32000
TOK = B * S                      # 4096 tokens total
TPC = TOK // N_CORES             # 512 tokens per core
P = 128                          # SBUF partitions
Q = TPC // P                     # 4 token tiles per core (128 tokens each)
K = NUM_CHUNKS                   # 4 segments per tile
G = Q * K                        # 16 segments per core
CHW = V // NUM_CHUNKS            # 8000
PRESCALE = 0.25                  # host multiplies logits by this before fp8

F8 = mybir.dt.float8e3
BF16 = mybir.dt.bfloat16
F32 = mybir.dt.float32
EXP = mybir.ActivationFunctionType.Exp
MULT = mybir.AluOpType.mult
SUB = mybir.AluOpType.subtract
BYPASS = mybir.AluOpType.bypass

# Flavor schedule (all ops verified compilable: Pool does only plain
# tensor_tensor; stt+accum lives on DVE; exp+accum on ACT):
#   A : ACT A1,A2,A3; Pool D=t-s;       DVE W-stt
#   AD: ACT A1,A2,A3;                   DVE W1,W2 stt (no D)
#   B : ACT A1,A2;    Pool D;           DVE m2,m4,Zstt,W
#   C : ACT A1,A2;    Pool D, m2;       DVE m4,Zstt,W
FLAV = ['B', 'B', 'A', 'B', 'A', 'B', 'A', 'B', 'B', 'A',
        'B', 'A', 'B', 'A', 'A', 'A']
assert len(FLAV) == G

A_FLAVS = ('A', 'AD')

# acc column layout: [Zu 0:16 | Zv 16:32 | W 32:48 | W2(AD) 48:64 | Zce 64:80]
NSTAT = 80


def _build_nc(repeat=1, flav_order=None):
    nc = bass.Bass()
    ts = nc.dram_tensor("ts", [2, TPC, V], F8, kind="ExternalInput")
    st = nc.dram_tensor("stats", [P, NSTAT], F32, kind="ExternalOutput")

    ng = G * repeat
    base_flav = flav_order if flav_order is not None else FLAV
    flav = [base_flav[g % G] for g in range(ng)]
    # B-seg ordinal (segments with a Zce chain) for es/m24 rings
    bidx = []
    b = 0
    for f in flav:
        bidx.append(b)
        if f not in A_FLAVS:
            b += 1
    bseg = [g for g in range(ng) if flav[g] not in A_FLAVS]

    # --- engine op streams (program order) + 1-based position maps -----
    # ACT: A1 runs one segment ahead of A2/A3, but the cadence starts at
    # seg 1 so A2(0) lands second and the DVE/Pool chains ramp early.
    act_stream = [('A1aa', 0), ('A1ab', 0), ('A1b', 0), ('A2', 0)]
    if flav[0] in A_FLAVS:
        act_stream.append(('A3', 0))
    if ng > 1:
        act_stream.append(('A1', 1))
    for g in range(1, ng):
        if g + 1 < ng:
            act_stream.append(('A1', g + 1))
        act_stream.append(('A2', g))
        if flav[g] in A_FLAVS:
            act_stream.append(('A3', g))

    # Pool: D(g) asap; for C segs also m2(g) (after A2(g)).
    pool_stream = []
    for g in range(ng):
        if flav[g] != 'AD':
            pool_stream.append(('D', g))
        if flav[g] == 'C':
            pool_stream.append(('m2', g))

    # DVE: A: [W]; AD: [W1, W2]; B: [m2, m4, Zstt, W]; C: [W, m4, Zstt]
    dve_stream = []
    for g in range(ng):
        f = flav[g]
        if f == 'A':
            dve_stream.append(('W', g))
        elif f == 'AD':
            dve_stream.append(('W1', g))
            dve_stream.append(('W2', g))
        elif f == 'B':
            dve_stream.extend([('m2', g), ('m4', g), ('Zstt', g), ('W', g)])
        else:  # C
            dve_stream.extend([('W', g), ('m4', g), ('Zstt', g)])

    def posmap(stream):
        m = {}
        for i, op in enumerate(stream):
            m[op] = i + 1
        return m

    aP = posmap(act_stream)
    pPos = posmap(pool_stream)
    vPos = posmap(dve_stream)
    aE_total, pP_total, vD_total = len(act_stream), len(pool_stream), len(dve_stream)

    def a1_pos(g):
        return aP[('A1b', 0)] if g == 0 else aP[('A1', g)]

    def after_last_act(g):      # last ACT op of seg g
        return aP[('A3', g)] if flav[g] in A_FLAVS else aP[('A2', g)]

    def after_last_dve_w(g):    # the W product(s) of seg g on DVE
        return vPos[('W2', g)] if flav[g] == 'AD' else vPos[('W', g)]

    def waits_readers_of_bufs(g):
        """t/s buffer of seg g is free when ACT (A2/A3), Pool (D) and
        DVE (W, in-place over the t-region) are all past seg g."""
        aw = after_last_act(g)
        vw = after_last_dve_w(g)
        pw = pPos[('D', g)] if flav[g] != 'AD' else 0
        return aw, vw, pw

    def waits_readers_of_et(g):
        return after_last_dve_w(g), 0

    def waits_readers_of_es(g):
        """Readers of the es slot written at seg g (B/C chains only)."""
        vw, pw = 0, 0
        if flav[g] == 'B':
            vw = vPos[('Zstt', g)]
        elif flav[g] == 'C':
            vw = vPos[('Zstt', g)]
            pw = pPos[('m2', g)]
        return vw, pw

    with ExitStack() as ctx:
        bufs = [
            ctx.enter_context(nc.sbuf_tensor(f"buf{i}", [P, 2 * CHW], F8))
            for i in range(5)
        ]
        et = [
            ctx.enter_context(nc.sbuf_tensor(f"et{i}", [P, CHW], F8))
            for i in range(4)
        ]
        es = [
            ctx.enter_context(nc.sbuf_tensor(f"es{i}", [P, CHW], BF16))
            for i in range(3)
        ]
        m24 = [
            ctx.enter_context(nc.sbuf_tensor(f"m24_{i}", [P, CHW], BF16))
            for i in range(2)
        ]
        acc = ctx.enter_context(nc.sbuf_tensor("acc", [P, NSTAT], F32))
        dTS = ctx.enter_context(nc.semaphore("dTS"))
        aE = ctx.enter_context(nc.semaphore("aE"))
        vD = ctx.enter_context(nc.semaphore("vD"))
        pP = ctx.enter_context(nc.semaphore("pP"))
        out_sem = ctx.enter_context(nc.semaphore("out_sem"))
        block = ctx.enter_context(nc.Block())

        def tseg(g):
            return bufs[g % 5][:, 0:CHW]

        def sseg(g):
            return bufs[g % 5][:, CHW:2 * CHW]

        def etb(g):
            return et[g % 4][:, :]

        def esb(g):
            return es[bidx[g] % 3][:, :]

        def m24b(g):
            return m24[bidx[g] % 2][:, :]

        def col(base, g):
            c = base * G + (g % G)
            return acc[:, c:c + 1]

        @block.sync
        def _(sync):
            for g in range(ng):
                q, k = (g % G) // K, g % K
                if g >= 5:
                    aw, vw, pw = waits_readers_of_bufs(g - 5)
                    sync.wait_ge(aE, aw)
                    sync.wait_ge(vD, vw)
                    if pw > 0:
                        sync.wait_ge(pP, pw)
                r0 = q * P
                src = ts[:, r0:r0 + P, k * CHW:(k + 1) * CHW].rearrange(
                    "a p v -> p a v")
                dst = bufs[g % 5][:].rearrange("p (a v) -> p a v", a=2)
                if g == 0:
                    # first load in four pieces: t[0:2000], t[2000:4000],
                    # t[4000:8000], s -- so A1aa can start ~3.6us in
                    q4, h2 = CHW // 4, CHW // 2
                    sync.dma_start(out=dst[:, 0:1, 0:q4],
                                   in_=src[:, 0:1, 0:q4]).then_inc(dTS, 16)
                    sync.dma_start(out=dst[:, 0:1, q4:h2],
                                   in_=src[:, 0:1, q4:h2]).then_inc(dTS, 16)
                    sync.dma_start(out=dst[:, 0:1, h2:CHW],
                                   in_=src[:, 0:1, h2:CHW]).then_inc(dTS, 16)
                    sync.dma_start(out=dst[:, 1:2, :],
                                   in_=src[:, 1:2, :]).then_inc(dTS, 16)
                else:
                    sync.dma_start(out=dst, in_=src).then_inc(dTS, 16)
            sync.wait_ge(aE, aE_total)
            sync.wait_ge(vD, vD_total)
            sync.wait_ge(pP, pP_total)
            sync.dma_start(out=st[:, :], in_=acc[:]).then_inc(out_sem, 16)
            sync.wait_ge(out_sem, 16)

        @block.scalar
        def _(scalar):
            HALF = CHW // 2
            QTR = CHW // 4
            for kind, g in act_stream:
                if kind == 'A1aa':
                    scalar.wait_ge(dTS, 16)
                    nc.scalar.activation(
                        etb(0)[:, 0:QTR], tseg(0)[:, 0:QTR], EXP,
                        bias=0.0, scale=0.8, accum_out=col(0, 0),
                    ).then_inc(aE, 1)
                elif kind == 'A1ab':
                    scalar.wait_ge(dTS, 32)
                    nc.scalar.activation(
                        etb(0)[:, QTR:HALF], tseg(0)[:, QTR:HALF], EXP,
                        bias=0.0, scale=0.8, accum_out=col(3, 0),
                    ).then_inc(aE, 1)
                elif kind == 'A1b':
                    scalar.wait_ge(dTS, 48)
                    nc.scalar.activation(
                        etb(0)[:, HALF:CHW], tseg(0)[:, HALF:CHW], EXP,
                        bias=0.0, scale=0.8, accum_out=col(3, 1),
                    ).then_inc(aE, 1)
                elif kind == 'A1':
                    scalar.wait_ge(dTS, 16 * (g + 4))
                    if g >= 4:
                        vw, pw = waits_readers_of_et(g - 4)
                        scalar.wait_ge(vD, vw)
                    nc.scalar.activation(
                        etb(g), tseg(g), EXP, bias=0.0, scale=0.8,
                        accum_out=col(0, g),
                    ).then_inc(aE, 1)
                elif kind == 'A2':
                    if g == 0:
                        scalar.wait_ge(dTS, 64)  # s-part of split load 0
                    bprev = bidx[g] - 3   # es ring depth 3
                    if bprev >= 0:
                        vw, pw = waits_readers_of_es(bseg[bprev])
                        if vw > 0:
                            scalar.wait_ge(vD, vw)
                        if pw > 0:
                            scalar.wait_ge(pP, pw)
                    nc.scalar.activation(
                        esb(g), sseg(g), EXP, bias=0.0, scale=0.8,
                        accum_out=col(1, g),
                    ).then_inc(aE, 1)
                else:  # A3
                    nc.scalar.activation(
                        esb(g), sseg(g), EXP, bias=0.0, scale=4.0,
                        accum_out=col(4, g),
                    ).then_inc(aE, 1)

        @block.vector
        def _(vector):
            for kind, g in dve_stream:
                if kind == 'W':
                    # (D byp 1) mult e_t, in place over the t-region.
                    # D(g) on Pool implies A1(g) (its own wait) and load.
                    vector.wait_ge(pP, pPos[('D', g)])
                    nc.vector.scalar_tensor_tensor(
                        out=tseg(g), in0=tseg(g), scalar=1.0,
                        in1=etb(g), op0=BYPASS, op1=MULT,
                        accum_out=col(2, g),
                    ).then_inc(vD, 1)
                elif kind == 'W1':
                    vector.wait_ge(aE, a1_pos(g))
                    nc.vector.scalar_tensor_tensor(
                        out=tseg(g), in0=etb(g), scalar=1.0,
                        in1=tseg(g), op0=BYPASS, op1=MULT,
                        accum_out=col(2, g),
                    ).then_inc(vD, 1)
                elif kind == 'W2':
                    nc.vector.scalar_tensor_tensor(
                        out=tseg(g), in0=etb(g), scalar=1.0,
                        in1=sseg(g), op0=BYPASS, op1=MULT,
                        accum_out=col(3, g),
                    ).then_inc(vD, 1)
                elif kind == 'm2':   # B only (C's m2 is on Pool)
                    vector.wait_ge(aE, aP[('A2', g)])
                    bp2 = bidx[g] - 2
                    if bp2 >= 0 and flav[bseg[bp2]] == 'C':
                        vector.wait_ge(pP, pPos[('m2', bseg[bp2])])
                    nc.vector.tensor_tensor(
                        out=m24b(g), in0=esb(g), in1=esb(g), op=MULT,
                    ).then_inc(vD, 1)
                elif kind == 'm4':
                    if flav[g] == 'C':
                        vector.wait_ge(pP, pPos[('m2', g)])
                    nc.vector.tensor_tensor(
                        out=m24b(g), in0=m24b(g), in1=m24b(g), op=MULT,
                    ).then_inc(vD, 1)
                else:  # Zstt
                    nc.vector.scalar_tensor_tensor(
                        out=m24b(g), in0=m24b(g), scalar=1.0,
                        in1=esb(g), op0=BYPASS, op1=MULT,
                        accum_out=col(4, g),
                    ).then_inc(vD, 1)

        @block.gpsimd
        def _(gp):
            for kind, g in pool_stream:
                if kind == 'D':
                    # D = t - s in place over the t-region (A1(g) read t)
                    if g == 0:
                        gp.wait_ge(dTS, 64)  # s-part of split load 0
                    gp.wait_ge(aE, a1_pos(g))
                    nc.gpsimd.tensor_tensor(
                        out=tseg(g), in0=tseg(g), in1=sseg(g), op=SUB,
                    ).then_inc(pP, 1)
                else:  # m2 for C segs
                    gp.wait_ge(aE, aP[('A2', g)])
                    bprev = bidx[g] - 2
                    if bprev >= 0:
                        f2 = flav[bseg[bprev]]
                        if f2 in ('B', 'C'):
                            gp.wait_ge(vD, vPos[('Zstt', bseg[bprev])])
                    nc.gpsimd.tensor_tensor(
                        out=m24b(g), in0=esb(g), in1=esb(g), op=MULT,
                    ).then_inc(pP, 1)

    return nc


_NC_CACHE = {}
last_results = None


def _get_nc(repeat=1):
    if repeat not in _NC_CACHE:
        _NC_CACHE[repeat] = _build_nc(repeat)
    return _NC_CACHE[repeat]


def _combine(results, s_full, lab):
    """Host-side float64 reduction of per-core [128, 80] stats -> loss."""
    # token = c*TPC + q*P + p ; segment g = 4q + chunk j
    w = np.empty((TOK, NUM_CHUNKS))
    zu = np.empty((TOK, NUM_CHUNKS))
    zv = np.empty((TOK, NUM_CHUNKS))
    zce = np.empty(TOK)

    def tokmajor(block):  # [P, G] -> [TPC, NUM_CHUNKS] in token order
        return block.reshape(P, Q, K).transpose(1, 0, 2).reshape(TPC, K)

    for c, r in enumerate(results):
        a = r["stats"].astype(np.float64)          # [128, 80]
        sl = slice(c * TPC, (c + 1) * TPC)
        zub = a[:, 0:G].copy()
        # seg 0 A1 split into three pieces; spare W2 cols of segs 0 and 1
        zub[:, 0] += a[:, 3 * G] + a[:, 3 * G + 1]
        zu[sl] = tokmajor(zub)
        zv[sl] = tokmajor(a[:, G:2 * G])
        wc = a[:, 2 * G:3 * G].copy()
        ad = np.array([f == 'AD' for f in FLAV])[None, :]
        wc = np.where(ad, wc - a[:, 3 * G:4 * G], wc)  # AD segs: W1 - W2
        w[sl] = tokmajor(wc)
        zce[sl] = tokmajor(a[:, 4 * G:5 * G]).sum(axis=1)

    # W stored = sum e_t*(t-s)/4 -> true sum e_t*(t-s) = 4*W
    kl = (4.0 * w) / (TEMP * zu) + np.log(zv) - np.log(zu)
    total_kl = kl.sum() * (TEMP * TEMP) * (CHW / V) / B

    s_label = s_full[np.arange(TOK), lab].astype(np.float64)
    nll = np.log(zce) - s_label
    valid = lab != PAD_ID
    n_valid = max(int(valid.sum()), 1)
    ce = float(nll[valid].sum()) / n_valid

    return ALPHA * total_kl + (1.0 - ALPHA) * ce


def kernel(student_logits, teacher_logits, labels):
    global last_results
    np_f8 = mybir.dt.np(F8)
    s_full = np.asarray(student_logits, dtype=np.float32).reshape(TOK, V)
    t_full = np.asarray(teacher_logits, dtype=np.float32).reshape(TOK, V)
    lab = np.asarray(labels).reshape(TOK).astype(np.int64)
    s_f8 = (s_full * PRESCALE).astype(np_f8)
    t_f8 = (t_full * PRESCALE).astype(np_f8)

    nc = _get_nc()
    in_maps = []
    for c in range(N_CORES):
        ts = np.ascontiguousarray(np.stack(
            [t_f8[c * TPC:(c + 1) * TPC], s_f8[c * TPC:(c + 1) * TPC]], axis=0))
        in_maps.append({"ts": ts})
    last_results = run_bass_kernel_spmd(nc, in_maps, core_ids=list(range(N_CORES)))
    loss = _combine(last_results.results, s_full, lab)
    return np.array(loss, dtype=np.float32)
